# revision 1
# baseline (speedup 1.0000x reference)
"""HGNN+LSTM kernel: data-parallel over batch B across 8 NeuronCores.

Self-contained: hardcodes problem shapes. Accepts FULL inputs, returns FULL
output. Sharding: batch B=32 -> 4 per core; all weights replicated. The edge
scatter-add is converted to dense adjacency matmuls (A built on host from the
tiny edge-index arrays; all heavy compute runs on-device).
"""
import numpy as np
import jax
import jax.numpy as jnp

NEG = 0.01
B, T, Nh, Nm = 32, 336, 100, 150
Fh, Fm, Hg, Hl, FUT = 8, 16, 64, 64, 24
NDEV = 8
BL = B // NDEV


def _fwd(dm, dh, A_h, A_m, W_rel_m, b_rel_m, W_root_m, W_rel_h, b_rel_h,
         W_root_h, W_ih, W_hh, bias, W_lin, b_lin):
    b = dh.shape[0]
    G = b * T
    xh = dh.reshape(G, Nh, Fh)
    xm = dm.reshape(G, Nm, Fm)

    # GraphConv aggregation as dense adjacency matmuls (edges fixed across G)
    agg_h = jnp.einsum('ns,gsf->gnf', A_h, xh)
    agg_m = jnp.einsum('ns,gsf->gnf', A_m, xm)
    out_h = agg_h @ W_rel_h.T + b_rel_h + xh @ W_root_h.T
    out_m = agg_m @ W_rel_m.T + b_rel_m + xh @ W_root_m.T
    x = jax.nn.leaky_relu(0.5 * (out_h + out_m), NEG)
    x = x.reshape(b, T, Nh, Hg).transpose(1, 0, 2, 3)  # [T, b, Nh, Hg]

    def step(carry, x_t):
        h, c = carry
        gates = (jnp.einsum('bnf,ngf->bng', x_t, W_ih)
                 + jnp.einsum('bnh,ngh->bng', h, W_hh) + bias)
        i, f, g, o = jnp.split(gates, 4, axis=-1)
        c = jax.nn.sigmoid(f) * c + jax.nn.sigmoid(i) * jnp.tanh(g)
        h = jax.nn.sigmoid(o) * jnp.tanh(c)
        return (h, c), None

    h0 = jnp.zeros((b, Nh, Hl), x.dtype)
    (h_last, _), _ = jax.lax.scan(step, (h0, h0), x)
    pred = h_last @ W_lin.T + b_lin
    return jax.nn.leaky_relu(pred, NEG)


_pfwd = jax.pmap(_fwd, in_axes=(0, 0) + (None,) * 13)


def kernel(**inputs):
    dm = np.asarray(inputs['data_meteo'])
    dh = np.asarray(inputs['data_hydro'])
    ei_h = np.asarray(inputs['hydro_edge_index'])
    ei_m = np.asarray(inputs['meteo_edge_index'])

    # Dense adjacency with duplicate-edge multiplicity: A[tgt, src] += 1
    A_h = np.zeros((Nh, Nh), np.float32)
    np.add.at(A_h, (ei_h[1], ei_h[0]), 1.0)
    A_m = np.zeros((Nh, Nm), np.float32)
    np.add.at(A_m, (ei_m[1], ei_m[0]), 1.0)

    dms = dm.reshape(NDEV, BL, T, Nm, Fm)
    dhs = dh.reshape(NDEV, BL, T, Nh, Fh)
    bias = np.asarray(inputs['b_ih']) + np.asarray(inputs['b_hh'])

    out = _pfwd(dms, dhs, A_h, A_m,
                np.asarray(inputs['W_rel_m']), np.asarray(inputs['b_rel_m']),
                np.asarray(inputs['W_root_m']),
                np.asarray(inputs['W_rel_h']), np.asarray(inputs['b_rel_h']),
                np.asarray(inputs['W_root_h']),
                np.asarray(inputs['W_ih']), np.asarray(inputs['W_hh']),
                bias,
                np.asarray(inputs['W_lin']), np.asarray(inputs['b_lin']))
    return np.asarray(out).reshape(B, Nh, FUT)

